# revision 1
# baseline (speedup 1.0000x reference)
"""Trainium2 Bass kernel for nn_LoraSequential (grouped LoRA + base GEMM).

Computes  y = concat_g[ (x_g @ A_g) @ B_g * 2 ]  +  x @ M   with
BATCH=4096, IN_F=OUT_F=4096, RANK=16, 8 equal segments.

Strategy: pure data parallelism over the 8 NeuronCores. Core g gets
segment g (512 tokens) and its own adapter pair (A_g, B_g) plus a full
copy of M — segments are disjoint so no collectives are needed. The
x shard is passed pre-transposed ([IN_F, SEG]) so the contraction dim
lands on SBUF partitions for both matmul operands.

Per-core compute: Y_g[512,4096] = X_g @ M + (2*H_g) @ B_g where
H_g^T = A_g^T-weighted matmul of X_g^T, accumulated in fp32 PSUM.
"""

import threading

import numpy as np

P = 128          # SBUF partitions / PE array size
BATCH = 4096
IN_F = 4096
OUT_F = 4096
RANK = 16
G = 8            # adapters == cores
SEG = BATCH // G         # 512 tokens per core
KT = IN_F // P           # 32 contraction tiles
TT = SEG // P            # 4 token tiles of 128
NB = 512                 # matmul moving-operand free dim (one PSUM bank)
OB = OUT_F // NB         # 8 output column blocks

_lock = threading.Lock()
_nc = None


def _build_nc():
    import concourse.bacc as bacc
    import concourse.mybir as mybir
    import concourse.tile as tile
    from concourse.bass import ts

    fp16 = mybir.dt.float16
    fp32 = mybir.dt.float32

    nc = bacc.Bacc(None, target_bir_lowering=False)
    xT = nc.dram_tensor("xT", [IN_F, SEG], fp16, kind="ExternalInput")
    # A is host-packed to [p, k, r] so its DMA uses contiguous 1KiB rows
    # per partition instead of 32B descriptors.
    A = nc.dram_tensor("A", [P, KT, RANK], fp16, kind="ExternalInput")
    B = nc.dram_tensor("B", [RANK, OUT_F], fp16, kind="ExternalInput")
    M = nc.dram_tensor("M", [IN_F, OUT_F], fp16, kind="ExternalInput")
    Y = nc.dram_tensor("Y", [SEG, OUT_F], fp16, kind="ExternalOutput")

    # [p, k, t] = xT[k*128+p, t]; partition dim = contraction chunk
    xT_r = xT.rearrange("(k p) t -> p k t", p=P)
    # [p, o, k, c] = M[k*128+p, o*512+c]
    M_r = M.rearrange("(k p) (o c) -> p o k c", p=P, c=NB)

    XCH = 8                # x^T prologue chunks
    KC = KT // XCH         # 4 k-tiles per chunk

    with tile.TileContext(nc) as tc:
        with (
            tc.tile_pool(name="const", bufs=1) as const,
            tc.tile_pool(name="mpool", bufs=3) as mpool,
            tc.tile_pool(name="opool", bufs=4) as opool,
            tc.tile_pool(name="pmain", bufs=6, space="PSUM") as pmain,
            tc.tile_pool(name="ph", bufs=1, space="PSUM") as phpool,
        ):
            # PE warm-up: the first real matmul can't start until ~10us of
            # preamble + input DMA; dummy matmuls on a zeroed scratch tile
            # need no data, fill that idle window, and flip the HAM clock
            # gate (1.2 -> 2.4 GHz) before real work arrives.
            warm_in = const.tile([P, NB], fp16)
            nc.gpsimd.memset(warm_in[:, :], 0.0)
            xT_s = const.tile([P, KT, SEG], fp16)
            A_s = const.tile([P, KT, RANK], fp16)
            # B and H^T padded to 128 partitions with zeros: a 16-row
            # LDWEIGHTS defeats the PE's pull-ahead and costs ~300ns at
            # every group boundary; a full-row weight load stays pipelined.
            # The extra 112 zero rows change nothing numerically.
            B_s = const.tile([P, OUT_F], fp16)
            HT_s = const.tile([P, SEG], fp16)
            nc.gpsimd.memset(B_s[:, :], 0.0)
            nc.gpsimd.memset(HT_s[:, :], 0.0)

            # Prologue: the HWDGE queue drains FIFO at ~350 GB/s, so
            # interleave x^T chunks with the first M slab; the PE streams H
            # matmuls as x^T chunks land, then main matmuls as slab chunks
            # land, instead of idling until everything arrives.
            m0_s = mpool.tile([P, KT, NB], fp16, tag="mslab", name="mslab_0")
            nc.sync.dma_start(out=A_s, in_=A[:, :, :])
            for c in range(XCH):
                nc.sync.dma_start(
                    out=xT_s[:, ts(c, KC), :], in_=xT_r[:, ts(c, KC), :]
                )
                nc.sync.dma_start(
                    out=m0_s[:, ts(c, KC), :], in_=M_r[:, 0, ts(c, KC), :]
                )
            nc.sync.dma_start(out=B_s[:RANK, :], in_=B[:, :])

            # Prologue compute, paced by the chunk DMAs above: for each
            # arriving (xT, m0) chunk pair, run its H matmuls AND the o=0
            # main-GEMM k-steps for all four t-tiles. This keeps the PE
            # saturated while the first 8.5 MiB stream in; H alone would
            # leave it idle half the time.
            pw = phpool.tile([P, NB], fp32, tag="warm", name="warm_ps")
            WARM = 12
            for i in range(WARM):
                nc.tensor.matmul(
                    pw,
                    lhsT=warm_in[:, :P],
                    rhs=warm_in,
                    start=(i == 0),
                    stop=(i == WARM - 1),
                )
            ph = phpool.tile([RANK, SEG], fp32)
            ps0 = [
                pmain.tile([P, NB], fp32, tag="ps", name=f"ps_0_{t}")
                for t in range(TT)
            ]
            for c in range(XCH):
                for k in range(c * KC, (c + 1) * KC):
                    nc.tensor.matmul(
                        ph,
                        lhsT=A_s[:, k, :],
                        rhs=xT_s[:, k, :],
                        start=(k == 0),
                        stop=(k == KT - 1),
                    )
                for t in range(TT):
                    for k in range(c * KC, (c + 1) * KC):
                        nc.tensor.matmul(
                            ps0[t],
                            lhsT=xT_s[:, k, ts(t, P)],
                            rhs=m0_s[:, k, :],
                            start=(k == 0),
                            stop=False,
                        )
            # Fold the *2.0 LoRA scale into the PSUM->SBUF eviction.
            nc.scalar.mul(HT_s[:RANK, :], ph, 2.0)
            for t in range(TT):
                # LoRA correction: (2H)^T.T @ B accumulates into the same
                # PSUM bank (K=16 partitions).
                nc.tensor.matmul(
                    ps0[t],
                    lhsT=HT_s[:, ts(t, P)],
                    rhs=B_s[:, ts(0, NB)],
                    start=False,
                    stop=True,
                )
                o_s = opool.tile([P, NB], fp16, tag="osb", name=f"osb_0_{t}")
                nc.vector.tensor_copy(out=o_s, in_=ps0[t])
                nc.sync.dma_start(out=Y[ts(t, P), ts(0, NB)], in_=o_s)

            for o in range(1, OB):
                m_s = mpool.tile([P, KT, NB], fp16, tag="mslab",
                                 name=f"mslab_{o}")
                nc.sync.dma_start(out=m_s, in_=M_r[:, o])
                for t in range(TT):
                    ps = pmain.tile([P, NB], fp32, tag="ps", name=f"ps_{o}_{t}")
                    for k in range(KT):
                        nc.tensor.matmul(
                            ps,
                            lhsT=xT_s[:, k, ts(t, P)],
                            rhs=m_s[:, k, :],
                            start=(k == 0),
                            stop=False,
                        )
                    nc.tensor.matmul(
                        ps,
                        lhsT=HT_s[:, ts(t, P)],
                        rhs=B_s[:, ts(o, NB)],
                        start=False,
                        stop=True,
                    )
                    o_s = opool.tile([P, NB], fp16, tag="osb", name=f"osb_{o}_{t}")
                    if o == OB - 1 and t == TT - 1:
                        # Last tile is on the critical tail: split the
                        # eviction so the first half's store overlaps the
                        # second half's PSUM->SBUF copy.
                        HB = NB // 2
                        for h in range(2):
                            nc.vector.tensor_copy(
                                out=o_s[:, ts(h, HB)], in_=ps[:, ts(h, HB)]
                            )
                            nc.sync.dma_start(
                                out=Y[ts(t, P), o * NB + h * HB : o * NB + (h + 1) * HB],
                                in_=o_s[:, ts(h, HB)],
                            )
                    else:
                        nc.vector.tensor_copy(out=o_s, in_=ps)
                        nc.sync.dma_start(out=Y[ts(t, P), ts(o, NB)], in_=o_s)
    nc.finalize()
    return nc


def get_nc():
    global _nc
    with _lock:
        if _nc is None:
            _nc = _build_nc()
        return _nc


def make_in_maps(x, lora_A, lora_B, M):
    x2 = np.ascontiguousarray(np.asarray(x, dtype=np.float16).reshape(BATCH, IN_F))
    lora_A = np.asarray(lora_A, dtype=np.float16)
    lora_B = np.asarray(lora_B, dtype=np.float16)
    M = np.ascontiguousarray(np.asarray(M, dtype=np.float16))
    in_maps = []
    for g in range(G):
        # pack A_g[i, r] -> [p, k, r] with i = k*128 + p
        a_packed = np.ascontiguousarray(
            lora_A[g].reshape(KT, P, RANK).transpose(1, 0, 2)
        )
        in_maps.append(
            {
                "xT": np.ascontiguousarray(x2[g * SEG : (g + 1) * SEG].T),
                "A": a_packed,
                "B": np.ascontiguousarray(lora_B[g]),
                "M": M,
            }
        )
    return in_maps


def kernel(x, lora_A, lora_B, M):
    from concourse.bass_utils import run_bass_kernel_spmd

    nc = get_nc()
    in_maps = make_in_maps(x, lora_A, lora_B, M)
    res = run_bass_kernel_spmd(nc, in_maps, core_ids=list(range(G))).results
    y = np.concatenate([r["Y"] for r in res], axis=0)
    return y.reshape(BATCH, 1, OUT_F)

